# revision 12
# baseline (speedup 1.0000x reference)
"""TRN2 Bass kernel for nn_DFT: out = log((x @ Wr.T)^2 + (x @ Wi.T)^2).

x: [262144, 256] f32;  dft_real/dft_imag: [256, 256] f32 (symmetric DFT mats).

Strategy
--------
Data-parallel over 8 NeuronCores: each core handles 32768 rows (frames).

Math: x is real, so the spectrum is conjugate-symmetric: mag[b, k] ==
mag[b, 256-k]. The device computes only k = 0..128 (129 unique columns);
the host mirrors the rest. Additionally Im X_0 == Im X_128 == 0, so row 0
of the imaginary weight block is dead and is repurposed to carry the
k=128 real row — 129 outputs from a single pair of 128-row matmul chains.

Layout: device works in transposed (frequency-major) orientation.
Host passes xT = x.T per core ([256, 32768], contiguous); the PE computes
psum[p, n] = sum_j W[j, p] * xT[j, n] with the contraction (j) on the
partition axis, i.e. plain matmuls with no on-chip transposes. The host
transposes the [129, 32768] per-core result back and mirrors columns
129..255 from 127..1.

Per 512-column group: 2 input DMAs, 4 accumulating matmuls (2 K-chunks
x {real, imag}), squares on ScalarE (reading PSUM), sum on VectorE, Ln on
ScalarE, 2 output DMAs.
"""

import numpy as np

NFFT = 256
BATCH = 262144
N_CORES = 8
B_CORE = BATCH // N_CORES  # 32768
NB = 512                   # moving-dim tile (fp32 matmul max, one PSUM bank)
NG = B_CORE // NB          # 64 groups
NOUT = NFFT // 2 + 1       # 129 unique spectrum columns

# "fp32": exact, PE at 4 cycles/row (2 half-rate passes per matmul).
#   Measured: 243 us HW, absmax 3.6e-4 vs the fp32 reference. PE-bound,
#   100% PE busy — at the fp32-mode roofline.
# "split3": hi/lo float32r decomposition, 3 full-rate passes — near-fp32
#   accuracy (drops only the lo*lo term). Measured: 251 us best, absmax
#   2.8e-2. The on-device hi/lo extraction costs ~190 us of VectorE time,
#   which starves the PE (HAM re-throttles). Offloading pieces to GpSimd
#   (casts: 380 us, mask-add: 312 us) or ScalarE (one cast: 280 us) only
#   made it worse — six engine arrangements measured, all lose to fp32.
MODE = "fold"

_PROG_CACHE = {}

# Input-chunk column widths for the fold mode: short head so the first
# matmul group's data lands fast, then uniform 1 MB-per-tensor bodies
# (DMA ~425 GB/s at that size) prefetched 2 chunks deep.
FOLD_WIDTHS = [1024, 3072] + [4096] * 7
assert sum(FOLD_WIDTHS) == B_CORE


def _build_program(mode):
    import concourse.bacc as bacc
    import concourse.mybir as mybir
    import concourse.tile as tile

    mm_dt = mybir.dt.float32
    f32 = mybir.dt.float32

    nc = bacc.Bacc("TRN2", target_bir_lowering=False, debug=False)
    if mode == "fold":
        return _build_fold(nc, mybir, tile)
    if mode == "fp16s3":
        outT = nc.dram_tensor("outT", [NOUT, B_CORE], f32, kind="ExternalOutput").ap()
        return _build_fp16s3(nc, mybir, tile, outT)
    xT = nc.dram_tensor("xT", [NFFT, B_CORE], mm_dt, kind="ExternalInput").ap()
    w = nc.dram_tensor("w", [NFFT, NFFT], mm_dt, kind="ExternalInput").ap()
    outT = nc.dram_tensor("outT", [NOUT, B_CORE], f32, kind="ExternalOutput").ap()

    if mode == "split3":
        return _build_split3(nc, mybir, tile, xT, w, outT)

    warmup = mode == "fp32w"
    tail_chunk = mode == "fp32t"
    Ln = mybir.ActivationFunctionType.Ln

    with tile.TileContext(nc) as tc:
        with (
            tc.tile_pool(name="wpool", bufs=1) as wpool,
            tc.tile_pool(name="xpool", bufs=4) as xpool,
            tc.tile_pool(name="pspool", bufs=4, space="PSUM") as pspool,
            tc.tile_pool(name="sqpool", bufs=4) as sqpool,
            tc.tile_pool(name="opool", bufs=4) as opool,
            tc.tile_pool(name="lpool", bufs=4) as lpool,
        ):
            # Weights resident for the whole kernel: w = [WrT | WiT'] with
            # rows j (contraction), cols 0:128 real / 128:256 imag.
            wt0 = wpool.tile([128, NFFT], mm_dt, tag="wt0")
            nc.sync.dma_start(wt0[:], w[0:128, :])
            wt1 = wpool.tile([128, NFFT], mm_dt, tag="wt1")
            nc.sync.dma_start(wt1[:], w[128:256, :])
            # Per-partition mask: 0 on partition 0 (whose imag slot carries
            # Re X_128, which must not leak into |X_0|^2), 1 elsewhere.
            mask = wpool.tile([128, 1], f32, tag="mask")
            nc.vector.memset(mask[:], 1.0)
            nc.vector.memset(mask[0:1, :], 0.0)

            if warmup:
                # Dummy matmuls on the weight tile, scheduled before the
                # first real matmul (they only depend on the wt0 DMA, which
                # lands ~4 us before x0). They trip the PE HAM activity
                # window so the real stream starts at 2.4 GHz instead of
                # ramping from 1.2 GHz ~3.4 us in.
                ps_w = pspool.tile([128, NB], f32, tag="ps_r")
                for _ in range(4):
                    nc.tensor.matmul(
                        ps_w[:, 0:NFFT], wt0[:, 0:128], wt0[:],
                        start=True, stop=True, skip_group_check=True,
                    )

            for g in range(NG):
                cs = bass_ts(g, NB)
                x0 = xpool.tile([128, NB], mm_dt, tag="x0")
                nc.sync.dma_start(x0[:], xT[0:128, cs])
                x1 = xpool.tile([128, NB], mm_dt, tag="x1")
                nc.sync.dma_start(x1[:], xT[128:256, cs])

                if tail_chunk and g == NG - 1:
                    # split the final group into two column halves so the
                    # first half's square/Ln/DMA chain overlaps the second
                    # half's matmuls, shortening the kernel tail.
                    ps_r = pspool.tile([128, NB], f32, tag="ps_r")
                    ps_i = pspool.tile([128, NB], f32, tag="ps_i")
                    sq_r = sqpool.tile([128, NB], f32, tag="sq_r")
                    sq_i = sqpool.tile([128, NB], f32, tag="sq_i")
                    sq_f = sqpool.tile([128, NB], f32, tag="sq_f")
                    o_main = opool.tile([128, NB], f32, tag="o_main")
                    o_last = lpool.tile([1, NB], f32, tag="o_last")
                    H = NB // 2
                    for c in range(2):
                        hs = bass_ts(c, H)
                        gcs = slice(g * NB + c * H, g * NB + (c + 1) * H)
                        nc.tensor.matmul(ps_r[:, hs], wt0[:, 0:128], x0[:, hs],
                                         start=True, stop=False, skip_group_check=True)
                        nc.tensor.matmul(ps_r[:, hs], wt1[:, 0:128], x1[:, hs],
                                         start=False, stop=True, skip_group_check=True)
                        nc.tensor.matmul(ps_i[:, hs], wt0[:, 128:256], x0[:, hs],
                                         start=True, stop=False, skip_group_check=True)
                        nc.tensor.matmul(ps_i[:, hs], wt1[:, 128:256], x1[:, hs],
                                         start=False, stop=True, skip_group_check=True)
                        nc.scalar.square(sq_r[:, hs], ps_r[:, hs])
                        nc.scalar.square(sq_i[:, hs], ps_i[:, hs])
                        nc.scalar.activation(o_last[:, hs], sq_i[0:1, hs], Ln)
                        nc.vector.scalar_tensor_tensor(
                            sq_f[:, hs], sq_i[:, hs], mask[:], sq_r[:, hs],
                            op0=mybir.AluOpType.mult, op1=mybir.AluOpType.add,
                        )
                        nc.scalar.activation(o_main[:, hs], sq_f[:, hs], Ln)
                        nc.sync.dma_start(outT[0:128, gcs], o_main[:, hs])
                        nc.sync.dma_start(outT[128:129, gcs], o_last[:, hs])
                    continue

                ps_r = pspool.tile([128, NB], f32, tag="ps_r")
                nc.tensor.matmul(ps_r[:], wt0[:, 0:128], x0[:], start=True, stop=False)
                nc.tensor.matmul(ps_r[:], wt1[:, 0:128], x1[:], start=False, stop=True)
                ps_i = pspool.tile([128, NB], f32, tag="ps_i")
                nc.tensor.matmul(ps_i[:], wt0[:, 128:256], x0[:], start=True, stop=False)
                nc.tensor.matmul(ps_i[:], wt1[:, 128:256], x1[:], start=False, stop=True)

                sq_r = sqpool.tile([128, NB], f32, tag="sq_r")
                nc.scalar.square(sq_r[:], ps_r[:])
                sq_i = sqpool.tile([128, NB], f32, tag="sq_i")
                nc.scalar.square(sq_i[:], ps_i[:])

                o_last = lpool.tile([1, NB], f32, tag="o_last")
                nc.scalar.activation(o_last[:], sq_i[0:1, :], Ln)

                # |X_k|^2 = r^2 + mask*i^2 (mask kills the repurposed row 0).
                sq_f = sqpool.tile([128, NB], f32, tag="sq_f")
                nc.vector.scalar_tensor_tensor(
                    sq_f[:], sq_i[:], mask[:], sq_r[:],
                    op0=mybir.AluOpType.mult, op1=mybir.AluOpType.add,
                )

                o_main = opool.tile([128, NB], f32, tag="o_main")
                nc.scalar.activation(o_main[:], sq_f[:], Ln)

                nc.sync.dma_start(outT[0:128, cs], o_main[:])
                nc.sync.dma_start(outT[128:129, cs], o_last[:])

    nc.compile()
    return nc


def _build_fold(nc, mybir, tile):
    """Even/odd-folded DFT: contraction halved from 256 to 128 per chain.

    Host ships E = [v, e'_1..e'_127] and O = [0, o_1..o_127] (each as an
    fp16 hi/lo pair), where e_j = x_j + x_{256-j}, o_j = x_j - x_{256-j},
    u = x_0 + x_128, v = x_0 - x_128, and e'_j = e_j - u for even j (the
    Dirichlet identity sum_{j even} cos(2pi jk/256) = -[k even] makes that
    fold exact). Then for k = 1..127:
        rxs[k] = v*[k odd] + sum_j e'_j cos(2pi jk/256)   (chain A)
        ixs[k] = sum_j o_j * (-sin(2pi jk/256))            (chain B)
    k = 0 and k = 128 are computed exactly on the host. Each chain is one
    128-contraction matmul per term; fp16 hi/lo 3-term split keeps ~22
    mantissa bits. Output log(rxs^2 + ixs^2) is written as fp16.
    """
    f32 = mybir.dt.float32
    f16 = mybir.dt.float16
    Ln = mybir.ActivationFunctionType.Ln

    names = ["eh", "el", "oh", "ol"]
    ins = {
        n: nc.dram_tensor(n, [128, B_CORE], f16, kind="ExternalInput").ap()
        for n in names
    }
    wpk = nc.dram_tensor("wpk", [128, 4 * 128], f16, kind="ExternalInput").ap()
    outT = nc.dram_tensor("outT", [128, B_CORE], f16, kind="ExternalOutput").ap()

    with tile.TileContext(nc) as tc:
        with (
            tc.tile_pool(name="wpool", bufs=1) as wpool,
            tc.tile_pool(name="xpool", bufs=3) as xpool,
            tc.tile_pool(name="pspool", bufs=4, space="PSUM") as pspool,
            tc.tile_pool(name="sqpool", bufs=4) as sqpool,
            tc.tile_pool(name="opool", bufs=2) as opool,
        ):
            bf16 = mybir.dt.bfloat16

            # HAM warmup on a memset tile: no DMA dependency, so the PE
            # activity window flips to 2.4 GHz before real data arrives.
            warm = wpool.tile([128, NB], f16, tag="warm")
            nc.vector.memset(warm[:], 0.0)
            ps_w = pspool.tile([128, NB], f32, tag="ps_a")
            for _ in range(4):
                nc.tensor.matmul(
                    ps_w[:], warm[:, 0:128], warm[:],
                    start=True, stop=True, skip_group_check=True,
                )

            wt = wpool.tile([128, 4 * 128], f16, tag="wt")
            nc.sync.dma_start(wt[:], wpk[:, :])
            wah, wal = wt[:, 0:128], wt[:, 128:256]
            wbh, wbl = wt[:, 256:384], wt[:, 384:512]

            def mm_a(ps_a, t, gsl):
                nc.tensor.matmul(ps_a[:], wah, t["eh"][:, gsl], start=True, stop=False)
                nc.tensor.matmul(ps_a[:], wah, t["el"][:, gsl], start=False, stop=False)
                nc.tensor.matmul(ps_a[:], wal, t["eh"][:, gsl], start=False, stop=True)

            def mm_b(ps_b, t, gsl):
                nc.tensor.matmul(ps_b[:], wbh, t["oh"][:, gsl], start=True, stop=False)
                nc.tensor.matmul(ps_b[:], wbh, t["ol"][:, gsl], start=False, stop=False)
                nc.tensor.matmul(ps_b[:], wbl, t["oh"][:, gsl], start=False, stop=True)

            def elementwise(ps_a, ps_b, ot, gsl):
                s1 = sqpool.tile([128, NB], bf16, tag="s1", name="s1")
                nc.scalar.square(s1[:], ps_a[:])
                s2 = sqpool.tile([128, NB], bf16, tag="s2", name="s2")
                nc.vector.tensor_copy(s2[:], ps_b[:])
                t2 = sqpool.tile([128, NB], bf16, tag="t2", name="t2")
                nc.vector.tensor_mul(t2[:], s2[:], s2[:])
                s3 = sqpool.tile([128, NB], bf16, tag="s3", name="s3")
                nc.vector.tensor_add(s3[:], t2[:], s1[:])
                nc.scalar.activation(ot[:, gsl], s3[:], Ln)

            col = 0
            for ci, w in enumerate(FOLD_WIDTHS):
                csl = slice(col, col + w)
                ng = w // NB
                t = {}
                # A-chain inputs first so group-0 matmuls can start after
                # half the chunk's bytes have landed.
                for n in names:
                    t[n] = xpool.tile([128, w], f16, tag=n, name=f"t_{n}")
                    nc.sync.dma_start(t[n][:], ins[n][:, csl])
                ot = opool.tile([128, w], f16, tag="ot")

                if ci == 0:
                    # head chunk: all A-chains before B-chains, so the PE
                    # starts as soon as eh/el land (ng*2 <= 8 PSUM banks).
                    pas = []
                    for g in range(ng):
                        gsl = bass_ts(g, NB)
                        ps_a = pspool.tile([128, NB], f32, tag="ps_a", name="ps_a")
                        mm_a(ps_a, t, gsl)
                        pas.append(ps_a)
                    for g in range(ng):
                        gsl = bass_ts(g, NB)
                        ps_b = pspool.tile([128, NB], f32, tag="ps_b", name="ps_b")
                        mm_b(ps_b, t, gsl)
                        elementwise(pas[g], ps_b, ot, gsl)
                else:
                    for g in range(ng):
                        gsl = bass_ts(g, NB)
                        ps_a = pspool.tile([128, NB], f32, tag="ps_a", name="ps_a")
                        mm_a(ps_a, t, gsl)
                        ps_b = pspool.tile([128, NB], f32, tag="ps_b", name="ps_b")
                        mm_b(ps_b, t, gsl)
                        elementwise(ps_a, ps_b, ot, gsl)

                # split out-DMAs: smoother HBM, earlier tail drain
                half = max(NB, w // 2)
                for o0 in range(0, w, half):
                    o1 = min(w, o0 + half)
                    nc.gpsimd.dma_start(
                        outT[:, col + o0 : col + o1], ot[:, o0:o1]
                    )
                col += w

    nc.compile()
    return nc


def _make_fold_weights(dft_real, dft_imag):
    """[128, 512] fp16: [WAh | WAl | WBh | WBl].

    WA row 0 = [k odd] (the v row), rows j=1..127 = dft_real[j, k];
    WB row 0 = 0,       rows j=1..127 = dft_imag[j, k]; col 0 unused (=0).
    """
    k = np.arange(128)
    WA = np.zeros((128, 128), dtype=np.float32)
    WA[0, :] = (k % 2).astype(np.float32)
    WA[0, 0] = 0.0
    WA[1:128, 1:128] = dft_real[1:128, 1:128]
    WB = np.zeros((128, 128), dtype=np.float32)
    WB[1:128, 1:128] = dft_imag[1:128, 1:128]
    out = []
    for W in (WA, WB):
        Wh = W.astype(np.float16)
        Wl = (W - Wh.astype(np.float32)).astype(np.float16)
        out += [Wh, Wl]
    return np.ascontiguousarray(np.concatenate(out, axis=1))


def _fold_core_inputs(xc):
    """xc [B_CORE, 256] f32 -> dict of four [128, B_CORE] fp16 tensors."""
    u = xc[:, 0] + xc[:, 128]
    v = xc[:, 0] - xc[:, 128]
    e = xc[:, 1:128] + xc[:, 255:128:-1]
    o = xc[:, 1:128] - xc[:, 255:128:-1]
    e[:, 1::2] -= u[:, None]  # columns j=2,4,..,126 (even j): e' = e - u
    E = np.concatenate([v[:, None], e], axis=1).T  # [128, B]
    O = np.concatenate([np.zeros((xc.shape[0], 1), np.float32), o], axis=1).T
    out = {}
    for name, M in (("e", np.ascontiguousarray(E)), ("o", np.ascontiguousarray(O))):
        h = M.astype(np.float16)
        l = (M - h.astype(np.float32)).astype(np.float16)
        out[name + "h"] = h
        out[name + "l"] = l
    return out


def _build_split3(nc, mybir, tile, xT, w, outT):
    """x = xh + xl, W = wh + wl (float32r hi/lo); r = xh*wh + xl*wh + xh*wl.

    float32r matmuls run a single full-rate pass (vs 2 half-rate passes for
    fp32), so 3 passes beat fp32's effective 4. The hi/lo products are exact
    in the fp32 accumulator; only the lo*lo term (~2^-22 relative) is lost.
    Splitting happens on-device so the exact fp32r rounding width is
    irrelevant: xh = hw_round(x), xl = hw_round(x - xh).
    """
    f32 = mybir.dt.float32
    f32r = mybir.dt.float32r
    Ln = mybir.ActivationFunctionType.Ln
    A = mybir.AluOpType

    with tile.TileContext(nc) as tc:
        with (
            tc.tile_pool(name="wpool", bufs=1) as wpool,
            tc.tile_pool(name="xpool", bufs=6) as xpool,
            tc.tile_pool(name="xspool", bufs=8) as xspool,
            tc.tile_pool(name="pspool", bufs=4, space="PSUM") as pspool,
            tc.tile_pool(name="sqpool", bufs=4) as sqpool,
            tc.tile_pool(name="opool", bufs=4) as opool,
        ):
            wf, wh, wl = [], [], []
            for kc in range(2):
                wf_t = wpool.tile([128, NFFT], f32, tag=f"wf{kc}")
                nc.sync.dma_start(wf_t[:], w[kc * 128 : (kc + 1) * 128, :])
                wh_t = wpool.tile([128, NFFT], f32r, tag=f"wh{kc}")
                nc.vector.tensor_copy(wh_t[:], wf_t[:])
                wl_t = wpool.tile([128, NFFT], f32r, tag=f"wl{kc}")
                nc.vector.tensor_sub(wl_t[:], wf_t[:], wh_t[:])
                wf.append(wf_t); wh.append(wh_t); wl.append(wl_t)

            mask = wpool.tile([128, 1], f32, tag="mask")
            nc.vector.memset(mask[:], 1.0)
            nc.vector.memset(mask[0:1, :], 0.0)

            coll = wpool.tile([NG, NB], f32, tag="coll")

            for g in range(NG):
                cs = bass_ts(g, NB)
                xh, xl = [], []
                for kc in range(2):
                    x_t = xpool.tile([128, NB], f32, tag=f"x{kc}")
                    nc.sync.dma_start(x_t[:], xT[kc * 128 : (kc + 1) * 128, cs])
                    xh_t = xspool.tile([128, NB], f32r, tag=f"xh{kc}")
                    nc.vector.tensor_copy(xh_t[:], x_t[:])
                    xl_t = xspool.tile([128, NB], f32r, tag=f"xl{kc}")
                    nc.vector.tensor_sub(xl_t[:], x_t[:], xh_t[:])
                    xh.append(xh_t); xl.append(xl_t)

                ps = []
                for half in range(2):  # 0: real, 1: imag
                    wcol = bass_ts(half, 128)
                    p = pspool.tile([128, NB], f32, tag=f"ps{half}")
                    terms = []
                    for kc in range(2):
                        terms += [
                            (wh[kc], xh[kc]),
                            (wh[kc], xl[kc]),
                            (wl[kc], xh[kc]),
                        ]
                    for t, (wt, xt) in enumerate(terms):
                        nc.tensor.matmul(
                            p[:], wt[:, wcol], xt[:],
                            start=(t == 0), stop=(t == len(terms) - 1),
                        )
                    ps.append(p)

                sq_r = sqpool.tile([128, NB], f32, tag="sq_r")
                nc.scalar.square(sq_r[:], ps[0][:])
                sq_i = sqpool.tile([128, NB], f32, tag="sq_i")
                nc.scalar.square(sq_i[:], ps[1][:])

                # stash Re(X_128)^2 (row 0 of sq_i) for the batched tail Ln.
                # DMA, not an engine copy: engine writes must start at a
                # 32-aligned partition; DMA can target partition g directly.
                nc.sync.dma_start(coll[g : g + 1, :], sq_i[0:1, :])
                sq_f = sqpool.tile([128, NB], f32, tag="sq_f")
                nc.vector.scalar_tensor_tensor(
                    sq_f[:], sq_i[:], mask[:], sq_r[:], op0=A.mult, op1=A.add
                )
                o_main = opool.tile([128, NB], f32, tag="o_main")
                nc.scalar.activation(o_main[:], sq_f[:], Ln)
                nc.sync.dma_start(outT[0:128, cs], o_main[:])

            o_coll = opool.tile([NG, NB], f32, tag="o_coll")
            nc.scalar.activation(o_coll[:], coll[:], Ln)
            out_last = outT[128:129, :].rearrange("a (g n) -> (a g) n", n=NB)
            nc.sync.dma_start(out_last, o_coll[:])

    nc.compile()
    return nc


def _build_fp16s3(nc, mybir, tile, outT):
    """Host-split fp16 hi/lo: r = xh*wh + xl*wh + xh*wl, all fp16 matmuls
    at 1 cycle/row. The split is exact on the host (IEEE fp16), costs zero
    device elementwise ops, and the same total DMA bytes as fp32 x."""
    f32 = mybir.dt.float32
    f16 = mybir.dt.float16
    Ln = mybir.ActivationFunctionType.Ln
    A = mybir.AluOpType

    xh_d = nc.dram_tensor("xh", [NFFT, B_CORE], f16, kind="ExternalInput").ap()
    xl_d = nc.dram_tensor("xl", [NFFT, B_CORE], f16, kind="ExternalInput").ap()
    wpk = nc.dram_tensor("wpk", [NFFT, 2 * NFFT], f16, kind="ExternalInput").ap()

    with tile.TileContext(nc) as tc:
        with (
            tc.tile_pool(name="wpool", bufs=1) as wpool,
            tc.tile_pool(name="xpool", bufs=6) as xpool,
            tc.tile_pool(name="pspool", bufs=4, space="PSUM") as pspool,
            tc.tile_pool(name="sqpool", bufs=4) as sqpool,
            tc.tile_pool(name="opool", bufs=4) as opool,
            tc.tile_pool(name="lpool", bufs=4) as lpool,
        ):
            wt = []
            for kc in range(2):
                w_t = wpool.tile([128, 2 * NFFT], f16, tag=f"wt{kc}")
                nc.sync.dma_start(w_t[:], wpk[kc * 128 : (kc + 1) * 128, :])
                wt.append(w_t)  # cols 0:256 = wh ([WrT|WiT']), 256:512 = wl

            mask = wpool.tile([128, 1], f32, tag="mask")
            nc.vector.memset(mask[:], 1.0)
            nc.vector.memset(mask[0:1, :], 0.0)

            for g in range(NG):
                cs = bass_ts(g, NB)
                xh, xl = [], []
                for kc in range(2):
                    ks = slice(kc * 128, (kc + 1) * 128)
                    xh_t = xpool.tile([128, NB], f16, tag=f"xh{kc}")
                    nc.sync.dma_start(xh_t[:], xh_d[ks, cs])
                    xl_t = xpool.tile([128, NB], f16, tag=f"xl{kc}")
                    nc.sync.dma_start(xl_t[:], xl_d[ks, cs])
                    xh.append(xh_t); xl.append(xl_t)

                ps = []
                for half in range(2):  # 0: real, 1: imag
                    wc_h = slice(half * 128, half * 128 + 128)          # wh cols
                    wc_l = slice(2 * NFFT // 2 + half * 128, 2 * NFFT // 2 + half * 128 + 128)  # wl cols
                    pt = pspool.tile([128, NB], f32, tag=f"ps{half}")
                    terms = []
                    for kc in range(2):
                        terms += [(wt[kc][:, wc_h], xh[kc]), (wt[kc][:, wc_h], xl[kc]),
                                  (wt[kc][:, wc_l], xh[kc])]
                    for t, (wap, xap) in enumerate(terms):
                        nc.tensor.matmul(pt[:], wap, xap[:],
                                         start=(t == 0), stop=(t == len(terms) - 1))
                    ps.append(pt)

                sq_r = sqpool.tile([128, NB], f32, tag="sq_r")
                nc.scalar.square(sq_r[:], ps[0][:])
                sq_i = sqpool.tile([128, NB], f32, tag="sq_i")
                nc.scalar.square(sq_i[:], ps[1][:])
                o_last = lpool.tile([1, NB], f32, tag="o_last")
                nc.scalar.activation(o_last[:], sq_i[0:1, :], Ln)
                sq_f = sqpool.tile([128, NB], f32, tag="sq_f")
                nc.vector.scalar_tensor_tensor(
                    sq_f[:], sq_i[:], mask[:], sq_r[:], op0=A.mult, op1=A.add
                )
                o_main = opool.tile([128, NB], f32, tag="o_main")
                nc.scalar.activation(o_main[:], sq_f[:], Ln)
                nc.sync.dma_start(outT[0:128, cs], o_main[:])
                nc.sync.dma_start(outT[128:129, cs], o_last[:])

    nc.compile()
    return nc


def bass_ts(i, size):
    return slice(i * size, (i + 1) * size)


def _get_program(mode):
    if mode not in _PROG_CACHE:
        _PROG_CACHE[mode] = _build_program(mode)
    return _PROG_CACHE[mode]


def _make_weights(dft_real, dft_imag):
    wr_half = dft_real[0:128, :]
    wi_half = dft_imag[0:128, :].copy()
    wi_half[0, :] = dft_real[128, :]  # dead Im X_0 row carries Re X_128
    return np.concatenate([wr_half.T, wi_half.T], axis=1).astype(np.float32)


def _run_fold(x, dft_real, dft_imag, trace=False, tmpdir=None):
    import concourse.bass_utils as bass_utils

    nc = _get_program("fold")
    wpk = _make_fold_weights(dft_real, dft_imag)
    in_maps = []
    for c in range(N_CORES):
        xc = np.ascontiguousarray(x[c * B_CORE : (c + 1) * B_CORE, :])
        m = _fold_core_inputs(xc)
        m["wpk"] = wpk
        in_maps.append(m)
    res = bass_utils.run_bass_kernel_spmd(
        nc, in_maps, core_ids=list(range(N_CORES)), trace=trace, tmpdir=tmpdir
    )
    full = np.empty((BATCH, NFFT), dtype=np.float32)
    for c in range(N_CORES):
        block = res.results[c]["outT"]  # [128, B_CORE] f16, row 0 unused
        full[c * B_CORE : (c + 1) * B_CORE, 1:128] = block[1:128].T.astype(np.float32)
    x64 = x.astype(np.float64)
    r0 = x64.sum(axis=1)
    r128 = x64[:, ::2].sum(axis=1) - x64[:, 1::2].sum(axis=1)
    full[:, 0] = np.log(r0 * r0).astype(np.float32)
    full[:, 128] = np.log(r128 * r128).astype(np.float32)
    full[:, 129:NFFT] = full[:, 127:0:-1]
    return full, res


def _run(x, dft_real, dft_imag, trace=False, tmpdir=None):
    import concourse.bass_utils as bass_utils

    if MODE == "fold":
        return _run_fold(x, dft_real, dft_imag, trace=trace, tmpdir=tmpdir)

    nc = _get_program(MODE)
    wfull = np.ascontiguousarray(_make_weights(dft_real, dft_imag))
    in_maps = []
    for c in range(N_CORES):
        xc = x[c * B_CORE : (c + 1) * B_CORE, :]
        xT_c = np.ascontiguousarray(xc.T)
        if MODE == "fp16s3":
            xh_c = xT_c.astype(np.float16)
            xl_c = (xT_c - xh_c.astype(np.float32)).astype(np.float16)
            wh = wfull.astype(np.float16)
            wl = (wfull - wh.astype(np.float32)).astype(np.float16)
            wpk = np.concatenate([wh, wl], axis=1)
            in_maps.append({"xh": xh_c, "xl": xl_c, "wpk": np.ascontiguousarray(wpk)})
        else:
            in_maps.append({"xT": xT_c, "w": wfull})
    res = bass_utils.run_bass_kernel_spmd(
        nc, in_maps, core_ids=list(range(N_CORES)), trace=trace, tmpdir=tmpdir
    )
    full = np.empty((BATCH, NFFT), dtype=np.float32)
    for c in range(N_CORES):
        block = res.results[c]["outT"]  # [129, B_CORE]
        full[c * B_CORE : (c + 1) * B_CORE, 0:NOUT] = block.T
    full[:, NOUT:NFFT] = full[:, NFFT - NOUT : 0 : -1]
    return full, res


def kernel(x, dft_real, dft_imag):
    x = np.asarray(x, dtype=np.float32)
    dft_real = np.asarray(dft_real, dtype=np.float32)
    dft_imag = np.asarray(dft_imag, dtype=np.float32)
    full, _ = _run(x, dft_real, dft_imag, trace=False)
    return full

